# revision 18
# baseline (speedup 1.0000x reference)
"""Masked dot-product attention on 8 TRN2 NeuronCores.

Math (per batch b):
    S = Q @ K^T / sqrt(64)                    [SQ, SK]
    S[:, k >= vl_b] = -1e6; A = softmax(S)    (masked cols -> weight 0)
    O = A @ V                                 [SQ, 64]

Device strategy (per core, SPMD — identical instruction stream):
  * scores are computed transposed: S_T[k, q] = sum_d K[k,d] Q[q,d]
    via matmul(lhsT=K^T tile [64,128], rhs=Q^T chunk [64,512]).
  * no max-subtraction: |S/8| <= ~6 so exp never overflows; the
    reference's masked lanes underflow to exactly 0 in fp32, we instead
    zero V rows (host-side) so masked keys contribute 0 to both
    numerator and denominator — identical result, zero device masking
    cost.
  * the exp over the [128, 1024] score tile is the throughput wall
    (ScalarE ACT runs 1 elem/cycle/lane at 1.2 GHz -> ~1.02us per unit,
    vs ~0.65us of PE time). So exp is SPLIT across two engines:
      - ScalarE units: exact exp via the ACT spline LUT (fp16 out).
      - VectorE units: one fused tensor_scalar (x*A + B -> int16,
        round-to-nearest): Schraudolph exp2 — the int16 result IS the
        fp16 bit pattern of 2^(x*log2e/8 + centering). Max rel err ~3%
        per element; softmax-normalized + averaged over many keys the
        end-to-end Frobenius error is ~1e-2 (gate 2e-2). Measured: the
        DVE convert rounds to nearest; constants account for that.
    Units are assigned greedily by projected engine busy-ns.
  * denominator via ones-column appended to V (host-side):
    O_aug^T[65, q] = sum_k V_aug[k, :]^T * exp(S_T[k, q]) accumulated in
    PSUM over k-tiles; row 64 is the softmax denominator.
  * host does final divide + transpose (tiny), so the device never
    needs cross-partition broadcasts.
  * matmul operands are fp16 (PE streams 2-byte dtypes at full rate;
    4-byte f32r measured 2.6x slower). PSUM accumulation stays fp32.

Work scheduling: the host knows valid_lens at compile time, so each core
receives a host-packed list of (q-chunk "slot", k-tile "unit") work items
covering only k < vl. All cores run the same program shape (same slot/unit
counts, compile-time constants); per-core differences live entirely in the
packed input data. Cores with fewer real k-tiles get padding units whose
V_aug is all-zero (contributes nothing).
"""

import functools
import math

import numpy as np

B, SQ, SK, D = 16, 2048, 2048, 64
NCORES = 8
KT = 128          # k rows per unit (one matmul stationary tile)
QCH = 1024        # q columns per slot
NSLOTS_TOTAL = B * (SQ // QCH)   # 32 slot-items across all cores
SLOTS_PER_CORE = NSLOTS_TOTAL // NCORES  # 4
VA_W = D + 1      # V columns + ones column
VA_P = KT         # V_aug padded to 128 cols: full-width mm2 keeps the PE
                  # array's HAM activity high (half-idle arrays throttle the
                  # clock to 1.2 GHz) and enables fast weight load
PW = KT + 2 * VA_P  # merged pair row width: K^T pair cols + 2x padded V_aug

# Schraudolph exp2-in-fp16-bits constants: for x = raw score (pre-1/sqrt(d)),
# approx fp16 bits y = rint(x * SCH_A + SCH_B); value(y) ~= exp(x/8).
# SCH_A = 1024 * log2(e) / 8; SCH_B = 15*1024 + 1024*shift with shift chosen
# to center the (1+f)/2^f ratio error (max +6.1% -> +-3.06%).
SCH_A = 1024.0 * math.log2(math.e) / 8.0
SCH_B = 15360.0 - 44.06

# emission-time greedy engine-balance costs (ns, from HW trace)
ACT_EXP_NS = 1018.0
DVE_EXP_NS = 1192.0
COPY_NS = 686.0
ACT_T0_NS = 1583.0  # table load + warm exp head start on ScalarE

_last_results = None  # stashed BassKernelResults for test.py introspection


def _nkt(vl: int) -> int:
    return max(1, min(SK // KT, math.ceil(vl / KT)))


def _make_schedule(vl: np.ndarray, full: bool = False):
    """Assign the 32 (batch, q-half) slot-items to 8 cores, balanced by
    k-tile count. An item may be SPLIT across slots/cores (partial-k
    attention sums are additive; the host sums partial outputs before
    dividing), which lets slot sizes drop below their group max with the
    overflow going to shared spill slots.

    Returns (slot_sizes, assign): slot_sizes[s] is the compile-time unit
    count of slot s (identical on every core); assign[core][s] is
    (batch, half, k_tile_start, n_real_ktiles) or None (pure padding)."""
    w = [SK // KT if full else _nkt(int(vl[b])) for b in range(B)]
    items = sorted(((b, h) for b in range(B) for h in range(SQ // QCH)),
                   key=lambda t: -w[t[0]])
    ngroups = len(items) // NCORES  # 4
    groups = [items[NCORES * s : NCORES * s + NCORES] for s in range(ngroups)]
    gmax = [max(w[b] for b, _ in g) for g in groups]
    gmin = [min(w[b] for b, _ in g) for g in groups]

    def evaluate(p):
        leftovers = []  # (len, batch, half, k_start)
        for s, g in enumerate(groups):
            for b, h in g:
                if w[b] > p[s]:
                    leftovers.append((w[b] - p[s], b, h, p[s]))
        leftovers.sort(key=lambda t: -t[0])
        spares = []
        for i in range(0, len(leftovers), NCORES):
            spares.append(leftovers[i : i + NCORES])
        spare_sizes = [chunk[0][0] for chunk in spares]
        return sum(p) + sum(spare_sizes), spares, spare_sizes

    import itertools
    best = None
    ranges = [range(gmin[s], gmax[s] + 1) for s in range(ngroups)]
    # keep the search tractable: only consider the top few reductions
    ranges = [r if len(r) <= 8 else range(gmax[s] - 7, gmax[s] + 1)
              for s, r in zip(range(ngroups), ranges)]
    for p in itertools.product(*ranges):
        total, spares, spare_sizes = evaluate(list(p))
        # each slot adds a pipeline-boundary stall worth ~0.7 units
        cost = total + 0.7 * (len(p) + len(spares))
        if best is None or cost < best[0]:
            best = (cost, list(p), spares, spare_sizes)
    _, p, spares, spare_sizes = best

    slot_sizes = list(p) + spare_sizes
    assign = [[None] * len(slot_sizes) for _ in range(NCORES)]
    for s, g in enumerate(groups):
        for c, (b, h) in enumerate(g):
            assign[c][s] = (b, h, 0, min(w[b], p[s]))
    for k, chunk in enumerate(spares):
        for c, (ln, b, h, k_start) in enumerate(chunk):
            assign[c][ngroups + k] = (b, h, k_start, ln)
    # order slots smallest-first: the small slots' pipeline-boundary bubbles
    # then coincide with the unavoidable HAM warm-up stalls at kernel start,
    # and the largest slot runs as one long saturated stretch at the end
    order = sorted(range(len(slot_sizes)), key=lambda s: slot_sizes[s])
    slot_sizes = [slot_sizes[s] for s in order]
    assign = [[a[s] for s in order] for a in assign]
    return tuple(slot_sizes), assign


@functools.lru_cache(maxsize=4)
def _build_program(slot_sizes: tuple):
    """Build + schedule the SPMD Bass program for the given slot shape."""
    import concourse.bacc as bacc
    import concourse.mybir as mybir
    import concourse.tile as tile

    n_units = sum(slot_sizes)
    f32 = mybir.dt.float32
    f16 = mybir.dt.float16
    i16 = mybir.dt.int16

    nc = bacc.Bacc(
        "TRN2",
        target_bir_lowering=False,
        debug=False,
        enable_asserts=False,
        num_devices=NCORES,
    )
    n_pairs = sum((u + 1) // 2 for u in slot_sizes)  # slot-local pairing
    n_slots = len(slot_sizes)
    qtd = nc.dram_tensor("qtd", [n_slots, KT, QCH], f16, kind="ExternalInput")
    uin = nc.dram_tensor("uin", [n_pairs, KT, PW], f16, kind="ExternalInput")
    o = nc.dram_tensor("o", [n_slots, VA_W, QCH], f32, kind="ExternalOutput")

    with tile.TileContext(nc) as tc:
        with (
            tc.tile_pool(name="qpool", bufs=3) as qpool,
            tc.tile_pool(name="upool", bufs=8) as upool,
            tc.tile_pool(name="ptpool", bufs=4) as ptpool,
            tc.tile_pool(name="opool", bufs=2) as opool,
            tc.tile_pool(name="scpool", bufs=1, space="PSUM") as scpool,
            tc.tile_pool(name="accpool", bufs=1, space="PSUM") as accpool,
        ):
            # Per pair of k-tile units (A, B): the 4 mm1 matmuls are emitted
            # interleaved (A-c0, B-c0, A-c1, B-c1) on PE row groups h0/h64 so
            # the two 64-deep contractions execute CONCURRENTLY in the array.
            # This both halves mm1 time and keeps array activity high enough
            # for the HAM clock gate to run the PE at full clock (a K=64
            # half-array stream alone stays throttled at 1.2 GHz).
            #
            # PE queue order is pinned to
            #   ... mm1-pair(p) -> mm2-pair(p-1) -> mm1-pair(p+1) ...
            # so the previous pair's mm2 fills the exp latency. Score tiles
            # rotate through 3 single-buffered PSUM tags (6 banks, +2 for the
            # accumulator = all 8), giving mm1 three units of WAR slack
            # against exp.
            scale = 1.0 / math.sqrt(D)
            exp_f = mybir.ActivationFunctionType.Exp
            # Dummy exp with no dependencies: pulls the ~2.7us ACT table
            # load into the DMA-priming phase instead of the first real exp.
            warm = qpool.tile([1, 8], f32, name="warm", tag="warm")
            nc.vector.memset(warm, 0.0)
            nc.scalar.activation(warm, warm, exp_f, scale=1.0)
            # PE warm-up: ~8 dummy matmuls (~3.4us at the cold 1.2GHz clock)
            # during the DMA-priming phase flip the HAM clock gate to 8/8
            # BEFORE the first real matmul, which otherwise runs the first
            # ~3.4us of real work at half clock.
            # NOTES from failed attempts: (1) 64-row half-array streams do
            # not register as busy in the HAM activity monitor — must use
            # full 128-row contractions; (2) memset-constant operands failed
            # to ramp (low datapath toggling and/or a power-capped run);
            # (3) reading the first Q tile works but its DMA lands ~10.4us
            # in (preamble + ~650ns/dma_start issue on the sync queue), too
            # late. GPSIMD iota needs no DMA and lands varied bit patterns
            # by ~7.5us.
            # (PE warm-up experiments: iota/memset data fails to flip the
            # HAM clock gate — the activity monitor tracks datapath toggling
            # and needs sign-varying gaussian-entropy operands. A qt-fed
            # warm-up works but the Q tile lands too late to pay off. See
            # WARM_DUMMIES to re-enable a gaussian warm-up fed by uin.)
            prev_wmm = None
            pending = []      # mm2 calls of the previous pair (emitted,
                              # ordering deferred until next pair's mm1s)
            prev_mm2_last = prev_wmm  # last dummy pins warm-up before mm1[0]
            gu = 0   # unit counter (sc-tag rotation)
            p_idx = 0  # global pair counter (uin index)
            # greedy exp/copy engine balance (emission-order projection)
            eng_busy = {"act": ACT_T0_NS, "dve": 0.0}
            nact = {"act": 0, "dve": 0}  # per-engine pt-tag rotation counters
            deferred_copies = []  # (acc, o_sb, slot, chunk, engine) to emit
            for s, nu in enumerate(slot_sizes):
                # Q^T chunk duplicated into both partition halves (h64 stream)
                qt = qpool.tile([KT, QCH], f16)
                nc.sync.dma_start(out=qt, in_=qtd[s])
                acc = accpool.tile([KT, QCH], f32)
                for jp in range((nu + 1) // 2):
                    ump = upool.tile([KT, PW], f16)
                    nc.sync.dma_start(out=ump, in_=uin[p_idx])
                    p_idx += 1
                    # A lone unit still gets a dummy row-group-B partner for
                    # mm1 (zero V_aug, no exp/mm2): a half-array matmul
                    # stream drops the HAM activity metric and re-throttles
                    # the PE clock to 1.2 GHz.
                    lone = 2 * jp + 1 >= nu
                    units = []
                    for half in (0, 1):
                        j = 2 * jp + half
                        real = not (lone and half == 1)
                        rows = slice(0, D) if half == 0 else slice(D, KT)
                        if real:
                            # engine choice: lower projected finish time
                            if (eng_busy["act"] + ACT_EXP_NS
                                    <= eng_busy["dve"] + DVE_EXP_NS):
                                eng = "act"
                                eng_busy["act"] += ACT_EXP_NS
                            else:
                                eng = "dve"
                                eng_busy["dve"] += DVE_EXP_NS
                            ptag = f"pt_{eng}{nact[eng] % 2}"
                            nact[eng] += 1
                            pt = ptpool.tile(
                                [KT, QCH], f16 if eng == "act" else i16,
                                name=f"pt_{gu}_{half}", tag=ptag)
                        else:
                            eng, pt = None, None
                        units.append((
                            j,
                            real,
                            ump[rows, 0:KT],                     # K^T tile
                            qt[rows, :],                          # Q^T stream
                            ump[:, KT + half * VA_P : KT + (half + 1) * VA_P],
                            scpool.tile([KT, QCH], f32, name=f"sc_{gu}_{half}",
                                        tag=f"sc{(gu + half) % 3}"),
                            pt,
                            eng,
                        ))
                    mm1 = []
                    nchunk = QCH // 512
                    for c in range(nchunk):
                        for j, real, kt_t, qt_h, va_t, sc, pt, eng in units:
                            mm1.append(nc.tensor.matmul(
                                sc[:, c * 512 : (c + 1) * 512],
                                lhsT=kt_t,
                                rhs=qt_h[:, c * 512 : (c + 1) * 512],
                                start=True,
                                stop=True,
                            ))
                            # emit each unit's exp right after its last mm1
                            # chunk so its ACT-queue wait lands per-exp (a
                            # trailing wait would gate exp-A on B's matmuls)
                            if c == nchunk - 1 and real:
                                if eng == "act":
                                    nc.scalar.activation(pt, sc, exp_f,
                                                         scale=scale)
                                else:
                                    nc.vector.tensor_scalar(
                                        pt[:, :], sc[:, :], SCH_A, SCH_B,
                                        mybir.AluOpType.mult,
                                        mybir.AluOpType.add)
                    # flush copies deferred from the previous slot AFTER this
                    # pair's exps are enqueued: engine queues are FIFO, so an
                    # earlier-emitted copy (waiting on the prior slot's last
                    # mm2) would stall this pair's exp behind it. The two
                    # chunks use separate o_sb tiles and one engine each so
                    # they run in PARALLEL (a shared tile serializes them on
                    # a tile-level WAW dep).
                    for acc_c, dst_c, o_idx, c_c, eng_c in deferred_copies:
                        src = acc_c[0:VA_W, c_c * 512 : (c_c + 1) * 512]
                        dst = dst_c[:, :]
                        eng_busy[eng_c] += COPY_NS
                        if eng_c == "act":
                            nc.scalar.activation(
                                dst, src, mybir.ActivationFunctionType.Copy)
                        else:
                            nc.vector.tensor_copy(dst, src)
                        nc.sync.dma_start(
                            out=o[o_idx, :, c_c * 512 : (c_c + 1) * 512],
                            in_=dst)
                    deferred_copies = []
                    if prev_mm2_last is not None:
                        tile.add_dep_helper(mm1[0].ins, prev_mm2_last.ins,
                                            False, "pe order")
                    for a, b in zip(mm1, mm1[1:]):
                        tile.add_dep_helper(b.ins, a.ins, False, "pe order")
                    for mm2 in pending:
                        tile.add_dep_helper(mm2.ins, mm1[-1].ins, False,
                                            "mm2 after next pair's mm1")
                    prev_mm2_last = pending[-1] if pending else prev_mm2_last
                    pending = []
                    for j, real, kt_t, qt_h, va_t, sc, pt, eng in units:
                        if not real:
                            continue
                        f16 = mybir.dt.float16
                        rhs_full = pt[:, :] if eng == "act" \
                            else pt[:, :].bitcast(f16)
                        for c in range(QCH // 512):
                            pending.append(nc.tensor.matmul(
                                acc[:, c * 512 : (c + 1) * 512],
                                lhsT=va_t,
                                rhs=rhs_full[:, c * 512 : (c + 1) * 512],
                                start=(j == 0),
                                stop=(j == nu - 1),
                            ))
                    for a, b in zip(pending, pending[1:]):
                        tile.add_dep_helper(b.ins, a.ins, False, "pe order")
                    gu += 2
                # copy + store per 512-col half; copies are deferred into the
                # next slot's first pair (see above) except at the very end.
                # One tile + one engine per chunk so both run concurrently.
                last = s == len(slot_sizes) - 1
                if not last:
                    for c in range(QCH // 512):
                        o_sb = opool.tile([VA_W, 512], f32, name=f"osb{s}_{c}",
                                          tag=f"osb{c}")
                        deferred_copies.append(
                            (acc, o_sb, s, c, "act" if c == 0 else "dve"))
                else:
                    # final slot: copy is on the critical tail; split into
                    # four 256-col chunks alternating engines so the two
                    # engines drain the accumulator in parallel.
                    for c in range(QCH // 256):
                        src = acc[0:VA_W, c * 256 : (c + 1) * 256]
                        o_sb = opool.tile([VA_W, 256], f32, name=f"osb{s}_{c}",
                                          tag=f"osbt{c}")
                        dst = o_sb[:, :]
                        if c % 2 == 1:
                            nc.scalar.activation(
                                dst, src, mybir.ActivationFunctionType.Copy)
                        else:
                            nc.vector.tensor_copy(dst, src)
                        nc.sync.dma_start(
                            out=o[s, :, c * 256 : (c + 1) * 256], in_=dst)
    nc.compile()
    return nc


def _pack_inputs(queries, keys, values, vl, slot_sizes, assign):
    """Build each core's packed device inputs per its schedule (mirrors the
    device program's slot-local pairing exactly)."""
    n_pairs = sum((u + 1) // 2 for u in slot_sizes)
    n_slots = len(slot_sizes)
    qT = np.ascontiguousarray(queries.transpose(0, 2, 1).astype(np.float16))
    kT = keys.astype(np.float16)  # [B, SK, D] row-major, sliced per k-tile
    in_maps = []
    for c in range(NCORES):
        qtd = np.zeros((n_slots, KT, QCH), np.float16)
        uin = np.zeros((n_pairs, KT, PW), np.float16)
        p_idx = 0
        for s, nu in enumerate(slot_sizes):
            if assign[c][s] is None:
                p_idx += (nu + 1) // 2
                continue  # pure-padding slot: all-zero inputs contribute 0
            b, h, ks, w = assign[c][s]
            qtd[s, :D] = qT[b, :, h * QCH : (h + 1) * QCH]
            qtd[s, D:KT] = qtd[s, :D]  # duplicate for the h64 row half
            nvalid = int(vl[b])
            for jp in range((nu + 1) // 2):
                for half in (0, 1):
                    # a lone unit's B half is a dummy mm1 partner (device
                    # skips its exp/mm2): real K data keeps array activity up
                    j = min(2 * jp + half, nu - 1)
                    t = ks + min(j, w - 1)  # padding units replay a k-tile
                    rows = slice(0, D) if half == 0 else slice(D, KT)
                    uin[p_idx, rows, :KT] = kT[b, t * KT : (t + 1) * KT, :].T
                    if j < w and not (half == 1 and 2 * jp + 1 >= nu):
                        k0 = t * KT
                        nv = min(max(nvalid - k0, 0), KT)
                        col0 = KT + half * VA_P
                        uin[p_idx, :nv, col0 : col0 + D] = values[b, k0 : k0 + nv, :]
                        uin[p_idx, :nv, col0 + D] = 1.0
                    # padding units leave V_aug zero -> contribute nothing
                p_idx += 1
        in_maps.append({"qtd": qtd, "uin": uin})
    return in_maps


def kernel(queries, keys, values, valid_lens, _full=False, _trace=False):
    global _last_results
    from concourse.bass_utils import run_bass_kernel_spmd

    queries = np.ascontiguousarray(np.asarray(queries, dtype=np.float32))
    keys = np.ascontiguousarray(np.asarray(keys, dtype=np.float32))
    values = np.ascontiguousarray(np.asarray(values, dtype=np.float32))
    vl = np.asarray(valid_lens).astype(np.int64).reshape(B)

    slot_sizes, assign = _make_schedule(vl, full=_full)
    nc = _build_program(slot_sizes)
    in_maps = _pack_inputs(queries, keys, values, vl, slot_sizes, assign)

    kwargs = {"trace": True} if _trace else {}
    res = run_bass_kernel_spmd(nc, in_maps, core_ids=list(range(NCORES)), **kwargs)
    _last_results = res

    # Sum partial (numerator, denominator) contributions per (batch, q-half),
    # then divide once — exact for split items.
    agg = np.zeros((B, SQ // QCH, VA_W, QCH), np.float64)
    for c in range(NCORES):
        o = res.results[c]["o"]  # [n_slots, VA_W, QCH]
        for s in range(len(slot_sizes)):
            if assign[c][s] is None:
                continue
            b, h, _, _ = assign[c][s]
            agg[b, h] += o[s]
    out = np.empty((B, SQ, D), np.float32)
    for b in range(B):
        for h in range(SQ // QCH):
            num = agg[b, h, :D, :]
            den = agg[b, h, D, :]
            out[b, h * QCH : (h + 1) * QCH, :] = (num / den).T.astype(np.float32)
    return out


# revision 19
# speedup vs baseline: 1.1546x; 1.1546x over previous
"""Masked dot-product attention on 8 TRN2 NeuronCores.

Math (per batch b):
    S = Q @ K^T / sqrt(64)                    [SQ, SK]
    S[:, k >= vl_b] = -1e6; A = softmax(S)    (masked cols -> weight 0)
    O = A @ V                                 [SQ, 64]

Device strategy (per core, SPMD — identical instruction stream):
  * scores are computed transposed: S_T[k, q] = sum_d K[k,d] Q[q,d]
    via matmul(lhsT=K^T tile [64,128], rhs=Q^T chunk [64,512]).
  * no max-subtraction: |S/8| <= ~6 so exp never overflows; the
    reference's masked lanes underflow to exactly 0 in fp32, we instead
    zero V rows (host-side) so masked keys contribute 0 to both
    numerator and denominator — identical result, zero device masking
    cost.
  * the exp over the [128, 1024] score tile is the throughput wall
    (ScalarE ACT runs 1 elem/cycle/lane at 1.2 GHz -> ~1.02us per unit,
    vs ~0.65us of PE time). So exp is SPLIT across two engines:
      - ScalarE units: exact exp via the ACT spline LUT (fp16 out).
      - VectorE units: one fused tensor_scalar (x*A + B -> int16,
        round-to-nearest): Schraudolph exp2 — the int16 result IS the
        fp16 bit pattern of 2^(x*log2e/8 + centering). Max rel err ~3%
        per element; softmax-normalized + averaged over many keys the
        end-to-end Frobenius error is ~1e-2 (gate 2e-2). Measured: the
        DVE convert rounds to nearest; constants account for that.
    Units are assigned greedily by projected engine busy-ns.
  * denominator via ones-column appended to V (host-side):
    O_aug^T[65, q] = sum_k V_aug[k, :]^T * exp(S_T[k, q]) accumulated in
    PSUM over k-tiles; row 64 is the softmax denominator.
  * host does final divide + transpose (tiny), so the device never
    needs cross-partition broadcasts.
  * matmul operands are fp16 (PE streams 2-byte dtypes at full rate;
    4-byte f32r measured 2.6x slower). PSUM accumulation stays fp32.

Work scheduling: the host knows valid_lens at compile time, so each core
receives a host-packed list of (q-chunk "slot", k-tile "unit") work items
covering only k < vl. All cores run the same program shape (same slot/unit
counts, compile-time constants); per-core differences live entirely in the
packed input data. Cores with fewer real k-tiles get padding units whose
V_aug is all-zero (contributes nothing).
"""

import functools
import math

import numpy as np

B, SQ, SK, D = 16, 2048, 2048, 64
NCORES = 8
KT = 128          # k rows per unit (one matmul stationary tile)
QCH = 1024        # q columns per slot
NSLOTS_TOTAL = B * (SQ // QCH)   # 32 slot-items across all cores
SLOTS_PER_CORE = NSLOTS_TOTAL // NCORES  # 4
VA_W = D + 1      # V columns + ones column
VA_P = KT         # V_aug padded to 128 cols: full-width mm2 keeps the PE
                  # array's HAM activity high (half-idle arrays throttle the
                  # clock to 1.2 GHz) and enables fast weight load
PW = KT + 2 * VA_P  # merged pair row width: K^T pair cols + 2x padded V_aug

# Schraudolph exp2-in-fp16-bits constants: for x = raw score (pre-1/sqrt(d)),
# approx fp16 bits y = rint(x * SCH_A + SCH_B); value(y) ~= exp(x/8).
# SCH_A = 1024 * log2(e) / 8; SCH_B = 15*1024 + 1024*shift with shift chosen
# to center the (1+f)/2^f ratio error (max +6.1% -> +-3.06%).
SCH_A = 1024.0 * math.log2(math.e) / 8.0
SCH_B = 15360.0 - 44.06

# emission-time greedy engine-balance costs (ns, from HW trace)
ACT_EXP_NS = 1018.0
DVE_EXP_NS = 1192.0
COPY_NS = 686.0
ACT_T0_NS = 1583.0  # table load + warm exp head start on ScalarE

_last_results = None  # stashed BassKernelResults for test.py introspection


def _nkt(vl: int) -> int:
    return max(1, min(SK // KT, math.ceil(vl / KT)))


def _make_schedule(vl: np.ndarray, full: bool = False):
    """Assign the 32 (batch, q-half) slot-items to 8 cores, balanced by
    k-tile count. An item may be SPLIT across slots/cores (partial-k
    attention sums are additive; the host sums partial outputs before
    dividing), which lets slot sizes drop below their group max with the
    overflow going to shared spill slots.

    Returns (slot_sizes, assign): slot_sizes[s] is the compile-time unit
    count of slot s (identical on every core); assign[core][s] is
    (batch, half, k_tile_start, n_real_ktiles) or None (pure padding)."""
    w = [SK // KT if full else _nkt(int(vl[b])) for b in range(B)]
    items = sorted(((b, h) for b in range(B) for h in range(SQ // QCH)),
                   key=lambda t: -w[t[0]])
    ngroups = len(items) // NCORES  # 4
    groups = [items[NCORES * s : NCORES * s + NCORES] for s in range(ngroups)]
    gmax = [max(w[b] for b, _ in g) for g in groups]
    gmin = [min(w[b] for b, _ in g) for g in groups]

    def evaluate(p):
        leftovers = []  # (len, batch, half, k_start)
        for s, g in enumerate(groups):
            for b, h in g:
                if w[b] > p[s]:
                    leftovers.append((w[b] - p[s], b, h, p[s]))
        leftovers.sort(key=lambda t: -t[0])
        spares = []
        for i in range(0, len(leftovers), NCORES):
            spares.append(leftovers[i : i + NCORES])
        spare_sizes = [chunk[0][0] for chunk in spares]
        return sum(p) + sum(spare_sizes), spares, spare_sizes

    import itertools
    best = None
    ranges = [range(gmin[s], gmax[s] + 1) for s in range(ngroups)]
    # keep the search tractable: only consider the top few reductions
    ranges = [r if len(r) <= 8 else range(gmax[s] - 7, gmax[s] + 1)
              for s, r in zip(range(ngroups), ranges)]
    for p in itertools.product(*ranges):
        total, spares, spare_sizes = evaluate(list(p))
        # each slot adds a pipeline-boundary stall worth ~0.7 units
        cost = total + 0.7 * (len(p) + len(spares))
        if best is None or cost < best[0]:
            best = (cost, list(p), spares, spare_sizes)
    _, p, spares, spare_sizes = best

    slot_sizes = list(p) + spare_sizes
    assign = [[None] * len(slot_sizes) for _ in range(NCORES)]
    for s, g in enumerate(groups):
        for c, (b, h) in enumerate(g):
            assign[c][s] = (b, h, 0, min(w[b], p[s]))
    for k, chunk in enumerate(spares):
        for c, (ln, b, h, k_start) in enumerate(chunk):
            assign[c][ngroups + k] = (b, h, k_start, ln)
    # order slots smallest-first: the small slots' pipeline-boundary bubbles
    # then coincide with the unavoidable HAM warm-up stalls at kernel start,
    # and the largest slot runs as one long saturated stretch at the end
    order = sorted(range(len(slot_sizes)), key=lambda s: slot_sizes[s])
    slot_sizes = [slot_sizes[s] for s in order]
    assign = [[a[s] for s in order] for a in assign]
    return tuple(slot_sizes), assign


@functools.lru_cache(maxsize=4)
def _build_program(slot_sizes: tuple):
    """Build + schedule the SPMD Bass program for the given slot shape."""
    import concourse.bacc as bacc
    import concourse.mybir as mybir
    import concourse.tile as tile

    n_units = sum(slot_sizes)
    f32 = mybir.dt.float32
    f16 = mybir.dt.float16
    i16 = mybir.dt.int16

    nc = bacc.Bacc(
        "TRN2",
        target_bir_lowering=False,
        debug=False,
        enable_asserts=False,
        num_devices=NCORES,
    )
    n_pairs = sum((u + 1) // 2 for u in slot_sizes)  # slot-local pairing
    n_slots = len(slot_sizes)
    qtd = nc.dram_tensor("qtd", [n_slots, KT, QCH], f16, kind="ExternalInput")
    uin = nc.dram_tensor("uin", [n_pairs, KT, PW], f16, kind="ExternalInput")
    o = nc.dram_tensor("o", [n_slots, VA_W, QCH], f32, kind="ExternalOutput")

    with tile.TileContext(nc) as tc:
        with (
            tc.tile_pool(name="qpool", bufs=3) as qpool,
            tc.tile_pool(name="upool", bufs=8) as upool,
            tc.tile_pool(name="ptpool", bufs=4) as ptpool,
            tc.tile_pool(name="opool", bufs=2) as opool,
            tc.tile_pool(name="scpool", bufs=1, space="PSUM") as scpool,
            tc.tile_pool(name="accpool", bufs=1, space="PSUM") as accpool,
        ):
            # Per pair of k-tile units (A, B): the 4 mm1 matmuls are emitted
            # interleaved (A-c0, B-c0, A-c1, B-c1) on PE row groups h0/h64 so
            # the two 64-deep contractions execute CONCURRENTLY in the array.
            # This both halves mm1 time and keeps array activity high enough
            # for the HAM clock gate to run the PE at full clock (a K=64
            # half-array stream alone stays throttled at 1.2 GHz).
            #
            # PE queue order is pinned to
            #   ... mm1-pair(p) -> mm2-pair(p-1) -> mm1-pair(p+1) ...
            # so the previous pair's mm2 fills the exp latency. Score tiles
            # rotate through 3 single-buffered PSUM tags (6 banks, +2 for the
            # accumulator = all 8), giving mm1 three units of WAR slack
            # against exp.
            scale = 1.0 / math.sqrt(D)
            exp_f = mybir.ActivationFunctionType.Exp
            # Dummy exp with no dependencies: pulls the ~2.7us ACT table
            # load into the DMA-priming phase instead of the first real exp.
            warm = qpool.tile([1, 8], f32, name="warm", tag="warm")
            nc.vector.memset(warm, 0.0)
            nc.scalar.activation(warm, warm, exp_f, scale=1.0)
            # No PE warm-up dummies: the HAM clock gate only ramps on
            # sign-varying high-entropy operands (iota/memset data measured
            # NOT to flip it — the activity monitor tracks datapath
            # toggling), and real input tiles land too late (~9-10us: fixed
            # preamble + ~650ns per dma_start issue on the sync queue) for
            # a pre-warm to beat simply starting real work cold.
            pending = []      # mm2 calls of the previous pair (emitted,
                              # ordering deferred until next pair's mm1s)
            prev_mm2_last = None  # last mm2 of the pair before that
            gu = 0   # unit counter (sc-tag rotation)
            p_idx = 0  # global pair counter (uin index)
            # greedy exp/copy engine balance (emission-order projection)
            eng_busy = {"act": ACT_T0_NS, "dve": 0.0}
            nact = {"act": 0, "dve": 0}  # per-engine pt-tag rotation counters
            deferred_copies = []  # (acc, o_sb, slot, chunk, engine) to emit
            for s, nu in enumerate(slot_sizes):
                # Q^T chunk duplicated into both partition halves (h64 stream)
                qt = qpool.tile([KT, QCH], f16)
                nc.sync.dma_start(out=qt, in_=qtd[s])
                acc = accpool.tile([KT, QCH], f32)
                for jp in range((nu + 1) // 2):
                    ump = upool.tile([KT, PW], f16)
                    nc.sync.dma_start(out=ump, in_=uin[p_idx])
                    p_idx += 1
                    # A lone unit still gets a dummy row-group-B partner for
                    # mm1 (zero V_aug, no exp/mm2): a half-array matmul
                    # stream drops the HAM activity metric and re-throttles
                    # the PE clock to 1.2 GHz.
                    lone = 2 * jp + 1 >= nu
                    units = []
                    for half in (0, 1):
                        j = 2 * jp + half
                        real = not (lone and half == 1)
                        rows = slice(0, D) if half == 0 else slice(D, KT)
                        if real:
                            # engine choice: lower projected finish time
                            if (eng_busy["act"] + ACT_EXP_NS
                                    <= eng_busy["dve"] + DVE_EXP_NS):
                                eng = "act"
                                eng_busy["act"] += ACT_EXP_NS
                            else:
                                eng = "dve"
                                eng_busy["dve"] += DVE_EXP_NS
                            ptag = f"pt_{eng}{nact[eng] % 2}"
                            nact[eng] += 1
                            pt = ptpool.tile(
                                [KT, QCH], f16 if eng == "act" else i16,
                                name=f"pt_{gu}_{half}", tag=ptag)
                        else:
                            eng, pt = None, None
                        units.append((
                            j,
                            real,
                            ump[rows, 0:KT],                     # K^T tile
                            qt[rows, :],                          # Q^T stream
                            ump[:, KT + half * VA_P : KT + (half + 1) * VA_P],
                            scpool.tile([KT, QCH], f32, name=f"sc_{gu}_{half}",
                                        tag=f"sc{(gu + half) % 3}"),
                            pt,
                            eng,
                        ))
                    mm1 = []
                    nchunk = QCH // 512
                    for c in range(nchunk):
                        for j, real, kt_t, qt_h, va_t, sc, pt, eng in units:
                            mm1.append(nc.tensor.matmul(
                                sc[:, c * 512 : (c + 1) * 512],
                                lhsT=kt_t,
                                rhs=qt_h[:, c * 512 : (c + 1) * 512],
                                start=True,
                                stop=True,
                            ))
                            # emit each unit's exp right after its last mm1
                            # chunk so its ACT-queue wait lands per-exp (a
                            # trailing wait would gate exp-A on B's matmuls)
                            if c == nchunk - 1 and real:
                                if eng == "act":
                                    nc.scalar.activation(pt, sc, exp_f,
                                                         scale=scale)
                                else:
                                    nc.vector.tensor_scalar(
                                        pt[:, :], sc[:, :], SCH_A, SCH_B,
                                        mybir.AluOpType.mult,
                                        mybir.AluOpType.add)
                    # flush copies deferred from the previous slot AFTER this
                    # pair's exps are enqueued: engine queues are FIFO, so an
                    # earlier-emitted copy (waiting on the prior slot's last
                    # mm2) would stall this pair's exp behind it. The two
                    # chunks use separate o_sb tiles and one engine each so
                    # they run in PARALLEL (a shared tile serializes them on
                    # a tile-level WAW dep).
                    for acc_c, dst_c, o_idx, c_c, eng_c in deferred_copies:
                        src = acc_c[0:VA_W, c_c * 512 : (c_c + 1) * 512]
                        dst = dst_c[:, :]
                        eng_busy[eng_c] += COPY_NS
                        if eng_c == "act":
                            nc.scalar.activation(
                                dst, src, mybir.ActivationFunctionType.Copy)
                        else:
                            nc.vector.tensor_copy(dst, src)
                        nc.sync.dma_start(
                            out=o[o_idx, :, c_c * 512 : (c_c + 1) * 512],
                            in_=dst)
                    deferred_copies = []
                    if prev_mm2_last is not None:
                        tile.add_dep_helper(mm1[0].ins, prev_mm2_last.ins,
                                            False, "pe order")
                    for a, b in zip(mm1, mm1[1:]):
                        tile.add_dep_helper(b.ins, a.ins, False, "pe order")
                    for mm2 in pending:
                        tile.add_dep_helper(mm2.ins, mm1[-1].ins, False,
                                            "mm2 after next pair's mm1")
                    prev_mm2_last = pending[-1] if pending else prev_mm2_last
                    pending = []
                    for j, real, kt_t, qt_h, va_t, sc, pt, eng in units:
                        if not real:
                            continue
                        f16 = mybir.dt.float16
                        rhs_full = pt[:, :] if eng == "act" \
                            else pt[:, :].bitcast(f16)
                        for c in range(QCH // 512):
                            pending.append(nc.tensor.matmul(
                                acc[:, c * 512 : (c + 1) * 512],
                                lhsT=va_t,
                                rhs=rhs_full[:, c * 512 : (c + 1) * 512],
                                start=(j == 0),
                                stop=(j == nu - 1),
                            ))
                    for a, b in zip(pending, pending[1:]):
                        tile.add_dep_helper(b.ins, a.ins, False, "pe order")
                    gu += 2
                # copy + store per 512-col half; copies are deferred into the
                # next slot's first pair (see above) except at the very end.
                # One tile + one engine per chunk so both run concurrently.
                last = s == len(slot_sizes) - 1
                if not last:
                    for c in range(QCH // 512):
                        o_sb = opool.tile([VA_W, 512], f32, name=f"osb{s}_{c}",
                                          tag=f"osb{c}")
                        deferred_copies.append(
                            (acc, o_sb, s, c, "act" if c == 0 else "dve"))
                else:
                    # final slot: copy is on the critical tail; split into
                    # four 256-col chunks alternating engines so the two
                    # engines drain the accumulator in parallel.
                    for c in range(QCH // 256):
                        src = acc[0:VA_W, c * 256 : (c + 1) * 256]
                        o_sb = opool.tile([VA_W, 256], f32, name=f"osb{s}_{c}",
                                          tag=f"osbt{c}")
                        dst = o_sb[:, :]
                        if c % 2 == 1:
                            nc.scalar.activation(
                                dst, src, mybir.ActivationFunctionType.Copy)
                        else:
                            nc.vector.tensor_copy(dst, src)
                        nc.sync.dma_start(
                            out=o[s, :, c * 256 : (c + 1) * 256], in_=dst)
    nc.compile()
    return nc


def _pack_inputs(queries, keys, values, vl, slot_sizes, assign):
    """Build each core's packed device inputs per its schedule (mirrors the
    device program's slot-local pairing exactly)."""
    n_pairs = sum((u + 1) // 2 for u in slot_sizes)
    n_slots = len(slot_sizes)
    qT = np.ascontiguousarray(queries.transpose(0, 2, 1).astype(np.float16))
    kT = keys.astype(np.float16)  # [B, SK, D] row-major, sliced per k-tile
    in_maps = []
    for c in range(NCORES):
        qtd = np.zeros((n_slots, KT, QCH), np.float16)
        uin = np.zeros((n_pairs, KT, PW), np.float16)
        p_idx = 0
        for s, nu in enumerate(slot_sizes):
            if assign[c][s] is None:
                p_idx += (nu + 1) // 2
                continue  # pure-padding slot: all-zero inputs contribute 0
            b, h, ks, w = assign[c][s]
            qtd[s, :D] = qT[b, :, h * QCH : (h + 1) * QCH]
            qtd[s, D:KT] = qtd[s, :D]  # duplicate for the h64 row half
            nvalid = int(vl[b])
            for jp in range((nu + 1) // 2):
                for half in (0, 1):
                    # a lone unit's B half is a dummy mm1 partner (device
                    # skips its exp/mm2): real K data keeps array activity up
                    j = min(2 * jp + half, nu - 1)
                    t = ks + min(j, w - 1)  # padding units replay a k-tile
                    rows = slice(0, D) if half == 0 else slice(D, KT)
                    uin[p_idx, rows, :KT] = kT[b, t * KT : (t + 1) * KT, :].T
                    if j < w and not (half == 1 and 2 * jp + 1 >= nu):
                        k0 = t * KT
                        nv = min(max(nvalid - k0, 0), KT)
                        col0 = KT + half * VA_P
                        uin[p_idx, :nv, col0 : col0 + D] = values[b, k0 : k0 + nv, :]
                        uin[p_idx, :nv, col0 + D] = 1.0
                    # padding units leave V_aug zero -> contribute nothing
                p_idx += 1
        in_maps.append({"qtd": qtd, "uin": uin})
    return in_maps


def kernel(queries, keys, values, valid_lens, _full=False, _trace=False):
    global _last_results
    from concourse.bass_utils import run_bass_kernel_spmd

    queries = np.ascontiguousarray(np.asarray(queries, dtype=np.float32))
    keys = np.ascontiguousarray(np.asarray(keys, dtype=np.float32))
    values = np.ascontiguousarray(np.asarray(values, dtype=np.float32))
    vl = np.asarray(valid_lens).astype(np.int64).reshape(B)

    slot_sizes, assign = _make_schedule(vl, full=_full)
    nc = _build_program(slot_sizes)
    in_maps = _pack_inputs(queries, keys, values, vl, slot_sizes, assign)

    kwargs = {"trace": True} if _trace else {}
    res = run_bass_kernel_spmd(nc, in_maps, core_ids=list(range(NCORES)), **kwargs)
    _last_results = res

    # Sum partial (numerator, denominator) contributions per (batch, q-half),
    # then divide once — exact for split items.
    agg = np.zeros((B, SQ // QCH, VA_W, QCH), np.float64)
    for c in range(NCORES):
        o = res.results[c]["o"]  # [n_slots, VA_W, QCH]
        for s in range(len(slot_sizes)):
            if assign[c][s] is None:
                continue
            b, h, _, _ = assign[c][s]
            agg[b, h] += o[s]
    out = np.empty((B, SQ, D), np.float32)
    for b in range(B):
        for h in range(SQ // QCH):
            num = agg[b, h, :D, :]
            den = agg[b, h, D, :]
            out[b, h * QCH : (h + 1) * QCH, :] = (num / den).T.astype(np.float32)
    return out


# revision 24
# speedup vs baseline: 1.1735x; 1.0163x over previous
"""Masked dot-product attention on 8 TRN2 NeuronCores.

Math (per batch b):
    S = Q @ K^T / sqrt(64)                    [SQ, SK]
    S[:, k >= vl_b] = -1e6; A = softmax(S)    (masked cols -> weight 0)
    O = A @ V                                 [SQ, 64]

Device strategy (per core, SPMD — identical instruction stream):
  * scores are computed transposed: S_T[k, q] = sum_d K[k,d] Q[q,d]
    via matmul(lhsT=K^T tile [64,128], rhs=Q^T chunk [64,512]).
  * no max-subtraction: |S/8| <= ~6 so exp never overflows; the
    reference's masked lanes underflow to exactly 0 in fp32, we instead
    zero V rows (host-side) so masked keys contribute 0 to both
    numerator and denominator — identical result, zero device masking
    cost.
  * the exp over the [128, 1024] score tile is the throughput wall
    (ScalarE ACT runs 1 elem/cycle/lane at 1.2 GHz -> ~1.02us per unit,
    vs ~0.65us of PE time). So exp is SPLIT across two engines:
      - ScalarE units: exact exp via the ACT spline LUT (fp16 out).
      - VectorE units: one fused tensor_scalar (x*A + B -> int16,
        round-to-nearest): Schraudolph exp2 — the int16 result IS the
        fp16 bit pattern of 2^(x*log2e/8 + centering). Max rel err ~3%
        per element; softmax-normalized + averaged over many keys the
        end-to-end Frobenius error is ~1e-2 (gate 2e-2). Measured: the
        DVE convert rounds to nearest; constants account for that.
    Units are assigned greedily by projected engine busy-ns.
  * denominator via ones-column appended to V (host-side):
    O_aug^T[65, q] = sum_k V_aug[k, :]^T * exp(S_T[k, q]) accumulated in
    PSUM over k-tiles; row 64 is the softmax denominator.
  * host does final divide + transpose (tiny), so the device never
    needs cross-partition broadcasts.
  * matmul operands are fp16 (PE streams 2-byte dtypes at full rate;
    4-byte f32r measured 2.6x slower). PSUM accumulation stays fp32.

Work scheduling: the host knows valid_lens at compile time, so each core
receives a host-packed list of (q-chunk "slot", k-tile "unit") work items
covering only k < vl. All cores run the same program shape (same slot/unit
counts, compile-time constants); per-core differences live entirely in the
packed input data. Cores with fewer real k-tiles get padding units whose
V_aug is all-zero (contributes nothing).
"""

import functools
import math

import numpy as np

B, SQ, SK, D = 16, 2048, 2048, 64
NCORES = 8
KT = 128          # k rows per unit (one matmul stationary tile)
QCH = 1024        # q columns per slot
NSLOTS_TOTAL = B * (SQ // QCH)   # 32 slot-items across all cores
SLOTS_PER_CORE = NSLOTS_TOTAL // NCORES  # 4
VA_W = D + 1      # V columns + ones column
VA_P = KT         # V_aug padded to 128 cols: full-width mm2 keeps the PE
                  # array's HAM activity high (half-idle arrays throttle the
                  # clock to 1.2 GHz) and enables fast weight load
PW = KT + 2 * VA_P  # merged pair row width: K^T pair cols + 2x padded V_aug

# Schraudolph exp2-in-fp16-bits constants: for x = raw score (pre-1/sqrt(d)),
# approx fp16 bits y = rint(x * SCH_A + SCH_B); value(y) ~= exp(x/8).
# SCH_A = 1024 * log2(e) / 8; SCH_B = 15*1024 + 1024*shift with shift chosen
# to center the (1+f)/2^f ratio error (max +6.1% -> +-3.06%).
SCH_A = 1024.0 * math.log2(math.e) / 8.0
SCH_B = 15360.0 - 44.06

# emission-time greedy engine-balance costs (ns, from HW trace)
ACT_EXP_NS = 1018.0
DVE_EXP_NS = 1192.0
COPY_NS = 686.0
ACT_T0_NS = 1583.0  # table load + warm exp head start on ScalarE

_last_results = None  # stashed BassKernelResults for test.py introspection


def _nkt(vl: int) -> int:
    return max(1, min(SK // KT, math.ceil(vl / KT)))


def _make_schedule(vl: np.ndarray, full: bool = False):
    """Assign the 32 (batch, q-half) slot-items to 8 cores, balanced by
    k-tile count. An item may be SPLIT across slots/cores (partial-k
    attention sums are additive; the host sums partial outputs before
    dividing), which lets slot sizes drop below their group max with the
    overflow going to shared spill slots.

    Returns (slot_sizes, assign): slot_sizes[s] is the compile-time unit
    count of slot s (identical on every core); assign[core][s] is
    (batch, half, k_tile_start, n_real_ktiles) or None (pure padding)."""
    w = [SK // KT if full else _nkt(int(vl[b])) for b in range(B)]
    items = sorted(((b, h) for b in range(B) for h in range(SQ // QCH)),
                   key=lambda t: -w[t[0]])
    ngroups = len(items) // NCORES  # 4
    groups = [items[NCORES * s : NCORES * s + NCORES] for s in range(ngroups)]
    gmax = [max(w[b] for b, _ in g) for g in groups]
    gmin = [min(w[b] for b, _ in g) for g in groups]

    def evaluate(p):
        leftovers = []  # (len, batch, half, k_start)
        for s, g in enumerate(groups):
            for b, h in g:
                if w[b] > p[s]:
                    leftovers.append((w[b] - p[s], b, h, p[s]))
        leftovers.sort(key=lambda t: -t[0])
        spares = []
        for i in range(0, len(leftovers), NCORES):
            spares.append(leftovers[i : i + NCORES])
        spare_sizes = [chunk[0][0] for chunk in spares]
        return sum(p) + sum(spare_sizes), spares, spare_sizes

    import itertools
    best = None
    ranges = [range(gmin[s], gmax[s] + 1) for s in range(ngroups)]
    # keep the search tractable: only consider the top few reductions
    ranges = [r if len(r) <= 8 else range(gmax[s] - 7, gmax[s] + 1)
              for s, r in zip(range(ngroups), ranges)]
    for p in itertools.product(*ranges):
        total, spares, spare_sizes = evaluate(list(p))
        # each slot adds a pipeline-boundary stall worth ~0.7 units
        cost = total + 0.7 * (len(p) + len(spares))
        if best is None or cost < best[0]:
            best = (cost, list(p), spares, spare_sizes)
    _, p, spares, spare_sizes = best

    slot_sizes = list(p) + spare_sizes
    assign = [[None] * len(slot_sizes) for _ in range(NCORES)]
    for s, g in enumerate(groups):
        for c, (b, h) in enumerate(g):
            assign[c][s] = (b, h, 0, min(w[b], p[s]))
    for k, chunk in enumerate(spares):
        for c, (ln, b, h, k_start) in enumerate(chunk):
            assign[c][ngroups + k] = (b, h, k_start, ln)
    # order slots smallest-first: the small slots' pipeline-boundary bubbles
    # then coincide with the unavoidable HAM warm-up stalls at kernel start,
    # and the largest slot runs as one long saturated stretch at the end
    order = sorted(range(len(slot_sizes)), key=lambda s: slot_sizes[s])
    slot_sizes = [slot_sizes[s] for s in order]
    assign = [[a[s] for s in order] for a in assign]
    return tuple(slot_sizes), assign


@functools.lru_cache(maxsize=4)
def _build_program(slot_sizes: tuple):
    """Build + schedule the SPMD Bass program for the given slot shape."""
    import concourse.bacc as bacc
    import concourse.mybir as mybir
    import concourse.tile as tile

    n_units = sum(slot_sizes)
    f32 = mybir.dt.float32
    f16 = mybir.dt.float16
    i16 = mybir.dt.int16

    nc = bacc.Bacc(
        "TRN2",
        target_bir_lowering=False,
        debug=False,
        enable_asserts=False,
        num_devices=NCORES,
    )
    n_pairs = sum((u + 1) // 2 for u in slot_sizes)  # slot-local pairing
    n_slots = len(slot_sizes)
    qtd = nc.dram_tensor("qtd", [n_slots, KT, QCH], f16, kind="ExternalInput")
    uin = nc.dram_tensor("uin", [n_pairs, KT, PW], f16, kind="ExternalInput")
    # outputs in fp16: num/den magnitudes fit comfortably (den <= ~4k,
    # f16 max 65504) and the ~5e-4 quantization is negligible vs the
    # Schraudolph term; halves the output DMA and the drain tail.
    o = nc.dram_tensor("o", [n_slots, VA_W, QCH], f16, kind="ExternalOutput")

    with tile.TileContext(nc) as tc:
        with (
            tc.tile_pool(name="qpool", bufs=3) as qpool,
            tc.tile_pool(name="upool", bufs=8) as upool,
            tc.tile_pool(name="ptpool", bufs=4) as ptpool,
            tc.tile_pool(name="opool", bufs=2) as opool,
            tc.tile_pool(name="scpool", bufs=1, space="PSUM") as scpool,
            tc.tile_pool(name="accpool", bufs=1, space="PSUM") as accpool,
        ):
            # Per pair of k-tile units (A, B): the 4 mm1 matmuls are emitted
            # interleaved (A-c0, B-c0, A-c1, B-c1) on PE row groups h0/h64 so
            # the two 64-deep contractions execute CONCURRENTLY in the array.
            # This both halves mm1 time and keeps array activity high enough
            # for the HAM clock gate to run the PE at full clock (a K=64
            # half-array stream alone stays throttled at 1.2 GHz).
            #
            # PE queue order is pinned to
            #   ... mm1-pair(p) -> mm2-pair(p-1) -> mm1-pair(p+1) ...
            # so the previous pair's mm2 fills the exp latency. Score tiles
            # rotate through 3 single-buffered PSUM tags (6 banks, +2 for the
            # accumulator = all 8), giving mm1 three units of WAR slack
            # against exp.
            scale = 1.0 / math.sqrt(D)
            exp_f = mybir.ActivationFunctionType.Exp
            # Dummy exp with no dependencies: pulls the ~2.7us ACT table
            # load into the DMA-priming phase instead of the first real exp.
            warm = qpool.tile([1, 8], f32, name="warm", tag="warm")
            nc.vector.memset(warm, 0.0)
            nc.scalar.activation(warm, warm, exp_f, scale=1.0)
            # No PE warm-up dummies: the HAM clock gate only ramps on
            # sign-varying high-entropy operands (iota/memset data measured
            # NOT to flip it — the activity monitor tracks datapath
            # toggling), and real input tiles land too late (~9-10us: fixed
            # preamble + ~650ns per dma_start issue on the sync queue) for
            # a pre-warm to beat simply starting real work cold.
            pending = []      # mm2 calls of the previous pair (emitted,
                              # ordering deferred until next pair's mm1s)
            prev_mm2_last = None  # last mm2 of the pair before that
            gu = 0   # unit counter (sc-tag rotation)
            p_idx = 0  # global pair counter (uin index)
            # greedy exp/copy engine balance (emission-order projection)
            eng_busy = {"act": ACT_T0_NS, "dve": 0.0}
            nact = {"act": 0, "dve": 0}  # per-engine pt-tag rotation counters
            deferred_copies = []  # (acc, o_sb, slot, chunk, engine) to emit
            for s, nu in enumerate(slot_sizes):
                # Q^T chunk duplicated into both partition halves (h64 stream)
                qt = qpool.tile([KT, QCH], f16)
                if s == 0:
                    # split the first Q tile's DMA so the c0-half (all the
                    # first matmuls need) lands in half the time — the head
                    # is serialized behind the fixed preamble, so this moves
                    # first-compute earlier by ~0.5-1us
                    nc.sync.dma_start(out=qt[:, 0:512], in_=qtd[s][:, 0:512])
                    nc.sync.dma_start(out=qt[:, 512:QCH],
                                      in_=qtd[s][:, 512:QCH])
                else:
                    nc.sync.dma_start(out=qt, in_=qtd[s])
                acc = accpool.tile([KT, QCH], f32)
                for jp in range((nu + 1) // 2):
                    ump = upool.tile([KT, PW], f16)
                    if p_idx == 0:
                        # split: the K^T part gates the first mm1; V parts
                        # are only needed ~1.2us later by the first mm2
                        nc.sync.dma_start(out=ump[:, 0:KT],
                                          in_=uin[0][:, 0:KT])
                        nc.sync.dma_start(out=ump[:, KT:PW],
                                          in_=uin[0][:, KT:PW])
                    else:
                        nc.sync.dma_start(out=ump, in_=uin[p_idx])
                    p_idx += 1
                    # A lone unit still gets a dummy row-group-B partner for
                    # mm1 (zero V_aug, no exp/mm2): a half-array matmul
                    # stream drops the HAM activity metric and re-throttles
                    # the PE clock to 1.2 GHz.
                    lone = 2 * jp + 1 >= nu
                    units = []
                    for half in (0, 1):
                        j = 2 * jp + half
                        real = not (lone and half == 1)
                        rows = slice(0, D) if half == 0 else slice(D, KT)
                        if real:
                            # engine choice: lower projected finish time
                            if (eng_busy["act"] + ACT_EXP_NS
                                    <= eng_busy["dve"] + DVE_EXP_NS):
                                eng = "act"
                                eng_busy["act"] += ACT_EXP_NS
                            else:
                                eng = "dve"
                                eng_busy["dve"] += DVE_EXP_NS
                            ptag = f"pt_{eng}{nact[eng] % 2}"
                            nact[eng] += 1
                            pt = ptpool.tile(
                                [KT, QCH], f16 if eng == "act" else i16,
                                name=f"pt_{gu}_{half}", tag=ptag)
                        else:
                            eng, pt = None, None
                        units.append((
                            j,
                            real,
                            ump[rows, 0:KT],                     # K^T tile
                            qt[rows, :],                          # Q^T stream
                            ump[:, KT + half * VA_P : KT + (half + 1) * VA_P],
                            scpool.tile([KT, QCH], f32, name=f"sc_{gu}_{half}",
                                        tag=f"sc{(gu + half) % 3}"),
                            pt,
                            eng,
                        ))
                    mm1 = []
                    nchunk = QCH // 512
                    for c in range(nchunk):
                        for j, real, kt_t, qt_h, va_t, sc, pt, eng in units:
                            mm1.append(nc.tensor.matmul(
                                sc[:, c * 512 : (c + 1) * 512],
                                lhsT=kt_t,
                                rhs=qt_h[:, c * 512 : (c + 1) * 512],
                                start=True,
                                stop=True,
                            ))
                            # emit each unit's exp right after its last mm1
                            # chunk so its ACT-queue wait lands per-exp (a
                            # trailing wait would gate exp-A on B's matmuls)
                            if c == nchunk - 1 and real:
                                if eng == "act":
                                    nc.scalar.activation(pt, sc, exp_f,
                                                         scale=scale)
                                else:
                                    nc.vector.tensor_scalar(
                                        pt[:, :], sc[:, :], SCH_A, SCH_B,
                                        mybir.AluOpType.mult,
                                        mybir.AluOpType.add)
                    # flush copies deferred from the previous slot AFTER this
                    # pair's exps are enqueued: engine queues are FIFO, so an
                    # earlier-emitted copy (waiting on the prior slot's last
                    # mm2) would stall this pair's exp behind it. The two
                    # chunks use separate o_sb tiles and one engine each so
                    # they run in PARALLEL (a shared tile serializes them on
                    # a tile-level WAW dep).
                    for acc_c, dst_c, o_idx, c_c, eng_c in deferred_copies:
                        src = acc_c[0:VA_W, c_c * 512 : (c_c + 1) * 512]
                        dst = dst_c[:, :]
                        eng_busy[eng_c] += COPY_NS
                        if eng_c == "act":
                            nc.scalar.activation(
                                dst, src, mybir.ActivationFunctionType.Copy)
                        else:
                            nc.vector.tensor_copy(dst, src)
                        nc.sync.dma_start(
                            out=o[o_idx, :, c_c * 512 : (c_c + 1) * 512],
                            in_=dst)
                    deferred_copies = []
                    if prev_mm2_last is not None:
                        tile.add_dep_helper(mm1[0].ins, prev_mm2_last.ins,
                                            False, "pe order")
                    for a, b in zip(mm1, mm1[1:]):
                        tile.add_dep_helper(b.ins, a.ins, False, "pe order")
                    for mm2 in pending:
                        tile.add_dep_helper(mm2.ins, mm1[-1].ins, False,
                                            "mm2 after next pair's mm1")
                    prev_mm2_last = pending[-1] if pending else prev_mm2_last
                    pending = []
                    for j, real, kt_t, qt_h, va_t, sc, pt, eng in units:
                        if not real:
                            continue
                        f16 = mybir.dt.float16
                        rhs_full = pt[:, :] if eng == "act" \
                            else pt[:, :].bitcast(f16)
                        for c in range(QCH // 512):
                            pending.append(nc.tensor.matmul(
                                acc[:, c * 512 : (c + 1) * 512],
                                lhsT=va_t,
                                rhs=rhs_full[:, c * 512 : (c + 1) * 512],
                                start=(j == 0),
                                stop=(j == nu - 1),
                            ))
                    for a, b in zip(pending, pending[1:]):
                        tile.add_dep_helper(b.ins, a.ins, False, "pe order")
                    gu += 2
                # copy + store per 512-col half; copies are deferred into the
                # next slot's first pair (see above) except at the very end.
                # One tile + one engine per chunk so both run concurrently.
                last = s == len(slot_sizes) - 1
                if not last:
                    for c in range(QCH // 512):
                        o_sb = opool.tile([VA_W, 512], f16, name=f"osb{s}_{c}",
                                          tag=f"osb{c}")
                        deferred_copies.append(
                            (acc, o_sb, s, c, "act" if c == 0 else "dve"))
                else:
                    # final slot: copy is on the critical tail; one 512-col
                    # chunk per engine, separate tiles, fully parallel.
                    # (A 4x256 split measured WORSE: the second op on each
                    # engine pays ~500ns of queue/sem gap, serializing.)
                    for c in range(QCH // 512):
                        src = acc[0:VA_W, c * 512 : (c + 1) * 512]
                        o_sb = opool.tile([VA_W, 512], f16, name=f"osb{s}_{c}",
                                          tag=f"osbt{c}")
                        dst = o_sb[:, :]
                        if c == 1:
                            nc.scalar.activation(
                                dst, src, mybir.ActivationFunctionType.Copy)
                        else:
                            nc.vector.tensor_copy(dst, src)
                        nc.sync.dma_start(
                            out=o[s, :, c * 512 : (c + 1) * 512], in_=dst)
    nc.compile()
    return nc


def _pack_inputs(queries, keys, values, vl, slot_sizes, assign):
    """Build each core's packed device inputs per its schedule (mirrors the
    device program's slot-local pairing exactly)."""
    n_pairs = sum((u + 1) // 2 for u in slot_sizes)
    n_slots = len(slot_sizes)
    qT = np.ascontiguousarray(queries.transpose(0, 2, 1).astype(np.float16))
    kT = keys.astype(np.float16)  # [B, SK, D] row-major, sliced per k-tile
    in_maps = []
    for c in range(NCORES):
        qtd = np.zeros((n_slots, KT, QCH), np.float16)
        uin = np.zeros((n_pairs, KT, PW), np.float16)
        p_idx = 0
        for s, nu in enumerate(slot_sizes):
            if assign[c][s] is None:
                p_idx += (nu + 1) // 2
                continue  # pure-padding slot: all-zero inputs contribute 0
            b, h, ks, w = assign[c][s]
            qtd[s, :D] = qT[b, :, h * QCH : (h + 1) * QCH]
            qtd[s, D:KT] = qtd[s, :D]  # duplicate for the h64 row half
            nvalid = int(vl[b])
            for jp in range((nu + 1) // 2):
                for half in (0, 1):
                    # a lone unit's B half is a dummy mm1 partner (device
                    # skips its exp/mm2): real K data keeps array activity up
                    j = min(2 * jp + half, nu - 1)
                    t = ks + min(j, w - 1)  # padding units replay a k-tile
                    rows = slice(0, D) if half == 0 else slice(D, KT)
                    uin[p_idx, rows, :KT] = kT[b, t * KT : (t + 1) * KT, :].T
                    if j < w and not (half == 1 and 2 * jp + 1 >= nu):
                        k0 = t * KT
                        nv = min(max(nvalid - k0, 0), KT)
                        col0 = KT + half * VA_P
                        uin[p_idx, :nv, col0 : col0 + D] = values[b, k0 : k0 + nv, :]
                        uin[p_idx, :nv, col0 + D] = 1.0
                    # padding units leave V_aug zero -> contribute nothing
                p_idx += 1
        in_maps.append({"qtd": qtd, "uin": uin})
    return in_maps


def kernel(queries, keys, values, valid_lens, _full=False, _trace=False):
    global _last_results
    from concourse.bass_utils import run_bass_kernel_spmd

    queries = np.ascontiguousarray(np.asarray(queries, dtype=np.float32))
    keys = np.ascontiguousarray(np.asarray(keys, dtype=np.float32))
    values = np.ascontiguousarray(np.asarray(values, dtype=np.float32))
    vl = np.asarray(valid_lens).astype(np.int64).reshape(B)

    slot_sizes, assign = _make_schedule(vl, full=_full)
    nc = _build_program(slot_sizes)
    in_maps = _pack_inputs(queries, keys, values, vl, slot_sizes, assign)

    kwargs = {"trace": True} if _trace else {}
    res = run_bass_kernel_spmd(nc, in_maps, core_ids=list(range(NCORES)), **kwargs)
    _last_results = res

    # Sum partial (numerator, denominator) contributions per (batch, q-half),
    # then divide once — exact for split items.
    agg = np.zeros((B, SQ // QCH, VA_W, QCH), np.float64)
    for c in range(NCORES):
        o = res.results[c]["o"]  # [n_slots, VA_W, QCH]
        for s in range(len(slot_sizes)):
            if assign[c][s] is None:
                continue
            b, h, _, _ = assign[c][s]
            agg[b, h] += o[s]
    out = np.empty((B, SQ, D), np.float32)
    for b in range(B):
        for h in range(SQ // QCH):
            num = agg[b, h, :D, :]
            den = agg[b, h, D, :]
            out[b, h * QCH : (h + 1) * QCH, :] = (num / den).T.astype(np.float32)
    return out


# revision 28
# speedup vs baseline: 1.2232x; 1.0424x over previous
"""Masked dot-product attention on 8 TRN2 NeuronCores.

Math (per batch b):
    S = Q @ K^T / sqrt(64)                    [SQ, SK]
    S[:, k >= vl_b] = -1e6; A = softmax(S)    (masked cols -> weight 0)
    O = A @ V                                 [SQ, 64]

Device strategy (per core, SPMD — identical instruction stream):
  * scores are computed transposed: S_T[k, q] = sum_d K[k,d] Q[q,d]
    via matmul(lhsT=K^T tile [64,128], rhs=Q^T chunk [64,512]).
  * no max-subtraction: |S/8| <= ~6 so exp never overflows; the
    reference's masked lanes underflow to exactly 0 in fp32, we instead
    zero V rows (host-side) so masked keys contribute 0 to both
    numerator and denominator — identical result, zero device masking
    cost.
  * the exp over the [128, 1024] score tile is the throughput wall
    (ScalarE ACT runs 1 elem/cycle/lane at 1.2 GHz -> ~1.02us per unit,
    vs ~0.65us of PE time). So exp is SPLIT across two engines:
      - ScalarE units: exact exp via the ACT spline LUT (fp16 out).
      - VectorE units: one fused tensor_scalar (x*A + B -> int16,
        round-to-nearest): Schraudolph exp2 — the int16 result IS the
        fp16 bit pattern of 2^(x*log2e/8 + centering). Max rel err ~3%
        per element; softmax-normalized + averaged over many keys the
        end-to-end Frobenius error is ~1e-2 (gate 2e-2). Measured: the
        DVE convert rounds to nearest; constants account for that.
    Units are assigned greedily by projected engine busy-ns.
  * denominator via ones-column appended to V (host-side):
    O_aug^T[65, q] = sum_k V_aug[k, :]^T * exp(S_T[k, q]) accumulated in
    PSUM over k-tiles; row 64 is the softmax denominator.
  * host does final divide + transpose (tiny), so the device never
    needs cross-partition broadcasts.
  * matmul operands are fp16 (PE streams 2-byte dtypes at full rate;
    4-byte f32r measured 2.6x slower). PSUM accumulation stays fp32.

Work scheduling: the host knows valid_lens at compile time, so each core
receives a host-packed list of (q-chunk "slot", k-tile "unit") work items
covering only k < vl. All cores run the same program shape (same slot/unit
counts, compile-time constants); per-core differences live entirely in the
packed input data. Cores with fewer real k-tiles get padding units whose
V_aug is all-zero (contributes nothing).
"""

import functools
import math

import numpy as np

B, SQ, SK, D = 16, 2048, 2048, 64
NCORES = 8
KT = 128          # k rows per unit (one matmul stationary tile)
QCH = 1024        # q columns per slot
NSLOTS_TOTAL = B * (SQ // QCH)   # 32 slot-items across all cores
SLOTS_PER_CORE = NSLOTS_TOTAL // NCORES  # 4
VA_W = D + 1      # V columns + ones column
VA_P = KT         # V_aug padded to 128 cols: full-width mm2 keeps the PE
                  # array's HAM activity high (half-idle arrays throttle the
                  # clock to 1.2 GHz) and enables fast weight load
PW = KT + 2 * VA_P  # merged pair row width: K^T pair cols + 2x padded V_aug

# Schraudolph exp2-in-fp16-bits constants: for x = raw score (pre-1/sqrt(d)),
# approx fp16 bits y = rint(x * SCH_A + SCH_B); value(y) ~= exp(x/8).
# SCH_A = 1024 * log2(e) / 8; SCH_B = 15*1024 + 1024*shift with shift chosen
# to center the (1+f)/2^f ratio error (max +6.1% -> +-3.06%).
SCH_A = 1024.0 * math.log2(math.e) / 8.0
SCH_B = 15360.0 - 44.06

# emission-time greedy engine-balance costs (ns, from HW trace)
ACT_EXP_NS = 1018.0
DVE_EXP_NS = 1192.0
COPY_NS = 686.0
ACT_T0_NS = 1583.0  # table load + warm exp head start on ScalarE

_last_results = None  # stashed BassKernelResults for test.py introspection


def _nkt(vl: int) -> int:
    return max(1, min(SK // KT, math.ceil(vl / KT)))


def _make_schedule(vl: np.ndarray, full: bool = False):
    """Assign the 32 (batch, q-half) slot-items to 8 cores, balanced by
    k-tile count. An item may be SPLIT across slots/cores (partial-k
    attention sums are additive; the host sums partial outputs before
    dividing), which lets slot sizes drop below their group max with the
    overflow going to shared spill slots.

    Returns (slot_sizes, assign): slot_sizes[s] is the compile-time unit
    count of slot s (identical on every core); assign[core][s] is
    (batch, half, k_tile_start, n_real_ktiles) or None (pure padding)."""
    w = [SK // KT if full else _nkt(int(vl[b])) for b in range(B)]
    items = sorted(((b, h) for b in range(B) for h in range(SQ // QCH)),
                   key=lambda t: -w[t[0]])
    ngroups = len(items) // NCORES  # 4
    groups = [items[NCORES * s : NCORES * s + NCORES] for s in range(ngroups)]
    gmax = [max(w[b] for b, _ in g) for g in groups]
    gmin = [min(w[b] for b, _ in g) for g in groups]

    def evaluate(p):
        leftovers = []  # (len, batch, half, k_start)
        for s, g in enumerate(groups):
            for b, h in g:
                if w[b] > p[s]:
                    leftovers.append((w[b] - p[s], b, h, p[s]))
        leftovers.sort(key=lambda t: -t[0])
        spares = []
        for i in range(0, len(leftovers), NCORES):
            spares.append(leftovers[i : i + NCORES])
        spare_sizes = [chunk[0][0] for chunk in spares]
        return sum(p) + sum(spare_sizes), spares, spare_sizes

    import itertools
    best = None
    ranges = [range(gmin[s], gmax[s] + 1) for s in range(ngroups)]
    # keep the search tractable: only consider the top few reductions
    ranges = [r if len(r) <= 8 else range(gmax[s] - 7, gmax[s] + 1)
              for s, r in zip(range(ngroups), ranges)]
    for p in itertools.product(*ranges):
        total, spares, spare_sizes = evaluate(list(p))
        # each slot adds a pipeline-boundary stall worth ~0.7 units
        cost = total + 0.7 * (len(p) + len(spares))
        if best is None or cost < best[0]:
            best = (cost, list(p), spares, spare_sizes)
    _, p, spares, spare_sizes = best

    slot_sizes = list(p) + spare_sizes
    assign = [[None] * len(slot_sizes) for _ in range(NCORES)]
    for s, g in enumerate(groups):
        for c, (b, h) in enumerate(g):
            assign[c][s] = (b, h, 0, min(w[b], p[s]))
    for k, chunk in enumerate(spares):
        for c, (ln, b, h, k_start) in enumerate(chunk):
            assign[c][ngroups + k] = (b, h, k_start, ln)
    # order slots smallest-first: the small slots' pipeline-boundary bubbles
    # then coincide with the unavoidable HAM warm-up stalls at kernel start,
    # and the largest slot runs as one long saturated stretch at the end
    order = sorted(range(len(slot_sizes)), key=lambda s: slot_sizes[s])
    slot_sizes = [slot_sizes[s] for s in order]
    assign = [[a[s] for s in order] for a in assign]
    return tuple(slot_sizes), assign


@functools.lru_cache(maxsize=4)
def _build_program(slot_sizes: tuple):
    """Build + schedule the SPMD Bass program for the given slot shape."""
    import concourse.bacc as bacc
    import concourse.mybir as mybir
    import concourse.tile as tile

    n_units = sum(slot_sizes)
    f32 = mybir.dt.float32
    f16 = mybir.dt.float16
    i16 = mybir.dt.int16

    nc = bacc.Bacc(
        "TRN2",
        target_bir_lowering=False,
        debug=False,
        enable_asserts=False,
        num_devices=NCORES,
    )
    n_pairs = sum((u + 1) // 2 for u in slot_sizes)  # slot-local pairing
    n_slots = len(slot_sizes)
    qtd = nc.dram_tensor("qtd", [n_slots, KT, QCH], f16, kind="ExternalInput")
    uin = nc.dram_tensor("uin", [n_pairs, KT, PW], f16, kind="ExternalInput")
    # outputs in fp16: num/den magnitudes fit comfortably (den <= ~4k,
    # f16 max 65504) and the ~5e-4 quantization is negligible vs the
    # Schraudolph term; halves the output DMA and the drain tail.
    o = nc.dram_tensor("o", [n_slots, VA_W, QCH], f16, kind="ExternalOutput")

    with tile.TileContext(nc) as tc:
        with (
            tc.tile_pool(name="qpool", bufs=3) as qpool,
            tc.tile_pool(name="upool", bufs=8) as upool,
            tc.tile_pool(name="ptpool", bufs=4) as ptpool,
            tc.tile_pool(name="opool", bufs=2) as opool,
            tc.tile_pool(name="scpool", bufs=1, space="PSUM") as scpool,
            tc.tile_pool(name="accpool", bufs=1, space="PSUM") as accpool,
        ):
            # Per pair of k-tile units (A, B): the 4 mm1 matmuls are emitted
            # interleaved (A-c0, B-c0, A-c1, B-c1) on PE row groups h0/h64 so
            # the two 64-deep contractions execute CONCURRENTLY in the array.
            # This both halves mm1 time and keeps array activity high enough
            # for the HAM clock gate to run the PE at full clock (a K=64
            # half-array stream alone stays throttled at 1.2 GHz).
            #
            # PE queue order is pinned to
            #   ... mm1-pair(p) -> mm2-pair(p-1) -> mm1-pair(p+1) ...
            # so the previous pair's mm2 fills the exp latency. Score tiles
            # rotate through 3 single-buffered PSUM tags (6 banks, +2 for the
            # accumulator = all 8), giving mm1 three units of WAR slack
            # against exp.
            scale = 1.0 / math.sqrt(D)
            exp_f = mybir.ActivationFunctionType.Exp
            # Dummy exp with no dependencies: pulls the ~2.7us ACT table
            # load into the DMA-priming phase instead of the first real exp.
            warm = qpool.tile([1, 8], f32, name="warm", tag="warm")
            nc.vector.memset(warm, 0.0)
            nc.scalar.activation(warm, warm, exp_f, scale=1.0)
            # No PE warm-up dummies: the HAM clock gate only ramps on
            # sign-varying high-entropy operands (iota/memset data measured
            # NOT to flip it — the activity monitor tracks datapath
            # toggling), and real input tiles land too late (~9-10us: fixed
            # preamble + ~650ns per dma_start issue on the sync queue) for
            # a pre-warm to beat simply starting real work cold.
            pending = []      # mm2 calls of the previous pair (emitted,
                              # ordering deferred until next pair's mm1s)
            prev_mm2_last = None  # last mm2 of the pair before that
            gu = 0   # unit counter (sc-tag rotation)
            p_idx = 0  # global pair counter (uin index)
            # greedy exp/copy engine balance (emission-order projection)
            eng_busy = {"act": ACT_T0_NS, "dve": 0.0}
            nact = {"act": 0, "dve": 0}  # per-engine pt-tag rotation counters
            deferred_copies = []  # (acc, o_sb, slot, chunk, engine) to emit
            for s, nu in enumerate(slot_sizes):
                # Q^T chunk duplicated into both partition halves (h64 stream)
                qt = qpool.tile([KT, QCH], f16)
                if s == 0:
                    # split the first Q tile's DMA so the c0-half (all the
                    # first matmuls need) lands in half the time — the head
                    # is serialized behind the fixed preamble, so this moves
                    # first-compute earlier by ~0.5-1us
                    nc.sync.dma_start(out=qt[:, 0:512], in_=qtd[s][:, 0:512])
                    nc.sync.dma_start(out=qt[:, 512:QCH],
                                      in_=qtd[s][:, 512:QCH])
                else:
                    nc.sync.dma_start(out=qt, in_=qtd[s])
                # acc ping-pongs across slots: the next slot's first mm2
                # then has NO WAR against this slot's output copies, which
                # drain in engine-FIFO slack instead of stalling the PE
                # ~1.5-2us per boundary (measured with a single acc buffer).
                # Paid for by dropping sc from 3 tags to 2 (PSUM: 2x2 sc +
                # 2x2 acc = 8 banks) — exp must now finish within one pair
                # period of its mm1, which holds with ~0.3-0.6us slack.
                acc = accpool.tile([KT, QCH], f32, name=f"acc{s}",
                                   tag=f"acc{s % 2}")
                for jp in range((nu + 1) // 2):
                    ump = upool.tile([KT, PW], f16)
                    if p_idx == 0:
                        # split: the K^T part gates the first mm1; V parts
                        # are only needed ~1.2us later by the first mm2
                        nc.sync.dma_start(out=ump[:, 0:KT],
                                          in_=uin[0][:, 0:KT])
                        nc.sync.dma_start(out=ump[:, KT:PW],
                                          in_=uin[0][:, KT:PW])
                    else:
                        nc.sync.dma_start(out=ump, in_=uin[p_idx])
                    p_idx += 1
                    # A lone unit still gets a dummy row-group-B partner for
                    # mm1 (zero V_aug, no exp/mm2): a half-array matmul
                    # stream drops the HAM activity metric and re-throttles
                    # the PE clock to 1.2 GHz.
                    lone = 2 * jp + 1 >= nu
                    units = []
                    for half in (0, 1):
                        j = 2 * jp + half
                        real = not (lone and half == 1)
                        rows = slice(0, D) if half == 0 else slice(D, KT)
                        if real:
                            # engine choice: lower projected finish time
                            if (eng_busy["act"] + ACT_EXP_NS
                                    <= eng_busy["dve"] + DVE_EXP_NS):
                                eng = "act"
                                eng_busy["act"] += ACT_EXP_NS
                            else:
                                eng = "dve"
                                eng_busy["dve"] += DVE_EXP_NS
                            ptag = f"pt_{eng}{nact[eng] % 2}"
                            nact[eng] += 1
                            pt = ptpool.tile(
                                [KT, QCH], f16 if eng == "act" else i16,
                                name=f"pt_{gu}_{half}", tag=ptag)
                        else:
                            eng, pt = None, None
                        units.append((
                            j,
                            real,
                            ump[rows, 0:KT],                     # K^T tile
                            qt[rows, :],                          # Q^T stream
                            ump[:, KT + half * VA_P : KT + (half + 1) * VA_P],
                            scpool.tile([KT, QCH], f32, name=f"sc_{gu}_{half}",
                                        tag=f"sc{(gu + half) % 2}"),
                            pt,
                            eng,
                        ))
                    mm1 = []
                    nchunk = QCH // 512
                    for c in range(nchunk):
                        for j, real, kt_t, qt_h, va_t, sc, pt, eng in units:
                            mm1.append(nc.tensor.matmul(
                                sc[:, c * 512 : (c + 1) * 512],
                                lhsT=kt_t,
                                rhs=qt_h[:, c * 512 : (c + 1) * 512],
                                start=True,
                                stop=True,
                            ))
                            # emit each unit's exp right after its last mm1
                            # chunk so its ACT-queue wait lands per-exp (a
                            # trailing wait would gate exp-A on B's matmuls)
                            if c == nchunk - 1 and real:
                                if eng == "act":
                                    nc.scalar.activation(pt, sc, exp_f,
                                                         scale=scale)
                                else:
                                    nc.vector.tensor_scalar(
                                        pt[:, :], sc[:, :], SCH_A, SCH_B,
                                        mybir.AluOpType.mult,
                                        mybir.AluOpType.add)
                    # flush copies deferred from the previous slot AFTER the
                    # second pair's exps of this slot are enqueued: engine
                    # queues are FIFO, so an earlier-emitted copy (waiting on
                    # the prior slot's last mm2) would stall exps behind it;
                    # two pairs deep they land in accumulated FIFO slack.
                    # One chunk per engine, separate tiles -> parallel.
                    if deferred_copies and jp == min(1, (nu + 1) // 2 - 1):
                        for acc_c, dst_c, o_idx, c_c, eng_c in deferred_copies:
                            src = acc_c[0:VA_W, c_c * 512 : (c_c + 1) * 512]
                            dst = dst_c[:, :]
                            eng_busy[eng_c] += COPY_NS
                            if eng_c == "act":
                                nc.scalar.activation(
                                    dst, src,
                                    mybir.ActivationFunctionType.Copy)
                            else:
                                nc.vector.tensor_copy(dst, src)
                            nc.sync.dma_start(
                                out=o[o_idx, :, c_c * 512 : (c_c + 1) * 512],
                                in_=dst)
                        deferred_copies = []
                    if prev_mm2_last is not None:
                        tile.add_dep_helper(mm1[0].ins, prev_mm2_last.ins,
                                            False, "pe order")
                    for a, b in zip(mm1, mm1[1:]):
                        tile.add_dep_helper(b.ins, a.ins, False, "pe order")
                    for mm2 in pending:
                        tile.add_dep_helper(mm2.ins, mm1[-1].ins, False,
                                            "mm2 after next pair's mm1")
                    prev_mm2_last = pending[-1] if pending else prev_mm2_last
                    pending = []
                    for j, real, kt_t, qt_h, va_t, sc, pt, eng in units:
                        if not real:
                            continue
                        f16 = mybir.dt.float16
                        rhs_full = pt[:, :] if eng == "act" \
                            else pt[:, :].bitcast(f16)
                        for c in range(QCH // 512):
                            pending.append(nc.tensor.matmul(
                                acc[:, c * 512 : (c + 1) * 512],
                                lhsT=va_t,
                                rhs=rhs_full[:, c * 512 : (c + 1) * 512],
                                start=(j == 0),
                                stop=(j == nu - 1),
                            ))
                    for a, b in zip(pending, pending[1:]):
                        tile.add_dep_helper(b.ins, a.ins, False, "pe order")
                    gu += 2
                # copy + store per 512-col half; copies are deferred into the
                # next slot's first pair (see above) except at the very end.
                # One tile + one engine per chunk so both run concurrently.
                last = s == len(slot_sizes) - 1
                if not last:
                    for c in range(QCH // 512):
                        o_sb = opool.tile([VA_W, 512], f16, name=f"osb{s}_{c}",
                                          tag=f"osb{c}")
                        deferred_copies.append(
                            (acc, o_sb, s, c, "act" if c == 0 else "dve"))
                else:
                    # final slot: copy is on the critical tail; one 512-col
                    # chunk per engine, separate tiles, fully parallel.
                    # (A 4x256 split measured WORSE: the second op on each
                    # engine pays ~500ns of queue/sem gap, serializing.)
                    for c in range(QCH // 512):
                        src = acc[0:VA_W, c * 512 : (c + 1) * 512]
                        o_sb = opool.tile([VA_W, 512], f16, name=f"osb{s}_{c}",
                                          tag=f"osbt{c}")
                        dst = o_sb[:, :]
                        if c == 1:
                            nc.scalar.activation(
                                dst, src, mybir.ActivationFunctionType.Copy)
                        else:
                            nc.vector.tensor_copy(dst, src)
                        nc.sync.dma_start(
                            out=o[s, :, c * 512 : (c + 1) * 512], in_=dst)
    nc.compile()
    return nc


def _pack_inputs(queries, keys, values, vl, slot_sizes, assign):
    """Build each core's packed device inputs per its schedule (mirrors the
    device program's slot-local pairing exactly)."""
    n_pairs = sum((u + 1) // 2 for u in slot_sizes)
    n_slots = len(slot_sizes)
    qT = np.ascontiguousarray(queries.transpose(0, 2, 1).astype(np.float16))
    kT = keys.astype(np.float16)  # [B, SK, D] row-major, sliced per k-tile
    in_maps = []
    for c in range(NCORES):
        qtd = np.zeros((n_slots, KT, QCH), np.float16)
        uin = np.zeros((n_pairs, KT, PW), np.float16)
        p_idx = 0
        for s, nu in enumerate(slot_sizes):
            if assign[c][s] is None:
                p_idx += (nu + 1) // 2
                continue  # pure-padding slot: all-zero inputs contribute 0
            b, h, ks, w = assign[c][s]
            qtd[s, :D] = qT[b, :, h * QCH : (h + 1) * QCH]
            qtd[s, D:KT] = qtd[s, :D]  # duplicate for the h64 row half
            nvalid = int(vl[b])
            for jp in range((nu + 1) // 2):
                for half in (0, 1):
                    # a lone unit's B half is a dummy mm1 partner (device
                    # skips its exp/mm2): real K data keeps array activity up
                    j = min(2 * jp + half, nu - 1)
                    t = ks + min(j, w - 1)  # padding units replay a k-tile
                    rows = slice(0, D) if half == 0 else slice(D, KT)
                    uin[p_idx, rows, :KT] = kT[b, t * KT : (t + 1) * KT, :].T
                    if j < w and not (half == 1 and 2 * jp + 1 >= nu):
                        k0 = t * KT
                        nv = min(max(nvalid - k0, 0), KT)
                        col0 = KT + half * VA_P
                        uin[p_idx, :nv, col0 : col0 + D] = values[b, k0 : k0 + nv, :]
                        uin[p_idx, :nv, col0 + D] = 1.0
                    # padding units leave V_aug zero -> contribute nothing
                p_idx += 1
        in_maps.append({"qtd": qtd, "uin": uin})
    return in_maps


def kernel(queries, keys, values, valid_lens, _full=False, _trace=False):
    global _last_results
    from concourse.bass_utils import run_bass_kernel_spmd

    queries = np.ascontiguousarray(np.asarray(queries, dtype=np.float32))
    keys = np.ascontiguousarray(np.asarray(keys, dtype=np.float32))
    values = np.ascontiguousarray(np.asarray(values, dtype=np.float32))
    vl = np.asarray(valid_lens).astype(np.int64).reshape(B)

    slot_sizes, assign = _make_schedule(vl, full=_full)
    nc = _build_program(slot_sizes)
    in_maps = _pack_inputs(queries, keys, values, vl, slot_sizes, assign)

    kwargs = {"trace": True} if _trace else {}
    res = run_bass_kernel_spmd(nc, in_maps, core_ids=list(range(NCORES)), **kwargs)
    _last_results = res

    # Sum partial (numerator, denominator) contributions per (batch, q-half),
    # then divide once — exact for split items.
    agg = np.zeros((B, SQ // QCH, VA_W, QCH), np.float64)
    for c in range(NCORES):
        o = res.results[c]["o"]  # [n_slots, VA_W, QCH]
        for s in range(len(slot_sizes)):
            if assign[c][s] is None:
                continue
            b, h, _, _ = assign[c][s]
            agg[b, h] += o[s]
    out = np.empty((B, SQ, D), np.float32)
    for b in range(B):
        for h in range(SQ // QCH):
            num = agg[b, h, :D, :]
            den = agg[b, h, D, :]
            out[b, h * QCH : (h + 1) * QCH, :] = (num / den).T.astype(np.float32)
    return out
